# revision 23
# baseline (speedup 1.0000x reference)
"""Batched single-qubit gate application on 8 TRN2 NeuronCores.

Problem: state (B=2048, N=8192) complex (separate f32 re/im planes), apply a
2x2 complex gate G on qubit 5 (pairs at stride R=128 within 256-blocks):
    out[b, l, c, r] = sum_a state[b, l, a, r] * G[a, c],  l<32, r<128.
Returns stacked (2, B, N) f32 [re, im].

Sharding: pure data parallel over the batch dim, 256 rows/core. The host
interleaves re/im at row granularity into one [256, 2, 8192] tensor per core
so every DMA touches all 128 SBUF partitions with a 2-dim DRAM access
pattern (64-partition or 3-dim-AP DMAs are much slower).

All device-side tensor data is fp16 (host casts f32 -> fp16 in, back out):
the kernel is HBM-bandwidth-bound, so halving the bytes halves the roofline
(~94us f32 -> ~47us fp16 at ~358 GB/s/core). fp16 rounding gives ~9e-4 max
rel err vs the 2e-2 gate; PSUM accumulation stays fp32.

Per-core strategy: ALL rows go through TensorE (fp16 matmul is 1 cyc/row vs
4 for f32, so PE covers the whole core's data in ~27us). The moving operand
keeps the natural DRAM row layout (interleaved [re_b; im_b] pairs on
partitions); stationary 128x128 matrices W(a,c) = kron(I64,
[[gr, gi], [-gi, gr]]) (host-built, passed as an input) turn each matmul
into "complex-scale 64 row-pairs by G[a,c]" with PSUM accumulating the two
a-terms. Work is organized as 16 chunks/rep of [128 rows x 2048 elems],
grouped into 8 PAIRS for DMA (1 MiB transfers -- SDMA engines only reach
line rate at ~1 MiB, and fewer DMAs means less fixed descriptor-path cost;
the 16 back-to-back matmuls per pair also keep the PE p-state hot):
  - PE: 8 matmuls per chunk (ls, c, a), PSUM set k&1 (2x2 banks,
    double-buffered at chunk granularity).
  - PSUM evacuation (f32 -> fp16 cast + (c,l,r)->(l,c,r) permute) alternates
    by pair between ScalarE (even pairs) and VectorE (odd pairs) so neither
    engine's copy throughput binds.
  - DMA issue is spread over three descriptor-generation paths: SP HWDGE
    ring carries most input pairs, ACT HWDGE carries the even-pair outputs,
    GPSIMD SWDGE carries every 4th input pair + the odd-pair outputs. No
    DVE 2-port perf-mode ops exist in the program, so SWDGE descriptor
    generation is never locked out of the shared SBUF port pair.

reps>1 builds the same pipeline repeated back-to-back in one NEFF (sems
keep counting) -- used only for steady-state hardware timing measurements.
"""

import sys

sys.path.insert(0, "/opt/trn_rl_repo")

from contextlib import ExitStack

import numpy as np

import concourse.bass as bass
import concourse.mybir as mybir
from concourse.bass_utils import run_bass_kernel_spmd

F32 = mybir.dt.float32
F16 = mybir.dt.float16

NCORES = 8
B = 2048
N = 8192
BC = B // NCORES  # 256 rows per core
JC = 2048  # j-chunk (elems per partition line)
NJ = N // JC  # 4
L_CHUNK = JC // 256  # 8 l-values per chunk
R = 128

_NC_CACHE = None


def _evac_is_act(k):
    # chunk k's PSUM evacuation engine: quads alternate ACT / DVE
    return ((k >> 2) & 1) == 0


def _cnt_evac(k, act):
    # chunks j <= k evacuated by ACT (act=True) or DVE
    return sum(1 for j in range(k + 1) if _evac_is_act(j) == act)


def _build_program(reps=1):
    nc = bass.Bass()

    sri = nc.declare_dram_parameter("sri", [BC, 2, N], F16, isOutput=False)
    wall = nc.declare_dram_parameter("wall", [128, 4, 128], F16, isOutput=False)
    opk = nc.declare_dram_parameter("opk", [BC, 2, N], F16, isOutput=True)

    # SBUF. DMAs move QUADS of chunks -- a quad is one row group x the full
    # 8192-elem width = a fully CONTIGUOUS 2 MiB DRAM region (peak SDMA
    # efficiency); compute and PSUM stay at chunk granularity (a chunk's
    # matmul output is half of PSUM). 4-deep input quad ring; buffer 3 (and
    # sem iP3) is exclusively GPSIMD's -- a SWDGE DMA's completion sem must
    # be 0 at its first use, so SWDGE and HWDGE incs can't share a sem.
    wsb = nc.alloc_sbuf_tensor("wsb", [128, 4, 128], F16)
    inP = [nc.alloc_sbuf_tensor(f"inP{s}", [128, 4 * JC], F16) for s in range(4)]
    stgA = [nc.alloc_sbuf_tensor(f"stgA{s}", [128, 4 * JC], F16) for s in range(2)]
    stgV = [nc.alloc_sbuf_tensor(f"stgV{s}", [128, 4 * JC], F16) for s in range(2)]
    # PSUM: 4 tensors x 2 banks = 8 banks; chunk k uses pair-set k&1.
    psp = [nc.alloc_psum_tensor(f"ps{i}", [128, 1024], F32) for i in range(4)]

    K = 16 * reps  # chunks
    P = 4 * reps  # quads; quad p = chunks 4p..4p+3 (one full row group)

    # moving-operand sub-lattice APs on quarter h of an input quad tile
    def lat(t, h, ls, a):
        return t[:].rearrange(
            "p (h ls l a r) -> p h ls l a r", h=4, ls=2, l=L_CHUNK // 2, a=2, r=R
        )[:, h, ls, :, a, :]

    # quarter h, ls-block of staging as [128, l, c, r] (paired evacuation)
    def lat_blk(t, h, ls):
        return t[:].rearrange(
            "p (h ls l a r) -> p h ls l a r", h=4, ls=2, l=L_CHUNK // 2, a=2, r=R
        )[:, h, ls, :, :, :]

    sri_flat = sri[:].rearrange("b e j -> (b e) j")
    opk_flat = opk[:].rearrange("b e j -> (b e) j")

    def prows(p):
        g = p % 4
        return slice(128 * g, 128 * g + 128)

    def pJ(p):
        return slice(0, N)

    with ExitStack() as _ctx:
        block = _ctx.enter_context(nc.Block())
        sem = {
            n: _ctx.enter_context(nc.semaphore(n))
            for n in [
                "wS", "mmS", "eA", "eV",
                "iP0", "iP1", "iP2", "iP3", "oA0", "oA1", "oV0", "oV1",
            ]
        }
        wS, mmS, eA, eV = sem["wS"], sem["mmS"], sem["eA"], sem["eV"]
        iP = [sem["iP0"], sem["iP1"], sem["iP2"], sem["iP3"]]
        oA = [sem["oA0"], sem["oA1"]]
        oV = [sem["oV0"], sem["oV1"]]

        def issue_in(eng, p):
            s = p % 4
            eng.dma_start(out=inP[s][:], in_=sri_flat[prows(p), pJ(p)]).then_inc(
                iP[s], 16
            )

        def psum_wait(eng, k):
            # psum pair-set k&1 free once evac of chunk k-2 is done
            if k >= 2:
                act = _evac_is_act(k - 2)
                eng.wait_ge(eA if act else eV, _cnt_evac(k - 2, act))

        @block.sync
        def _(sync):
            issue_in(sync, 1)
            issue_in(sync, 2)
            for p in range(4, P):
                if p % 4 == 3:
                    continue  # gpsimd's input quad
                # inP[p%4] is free once mm of quad p-4 is done
                sync.wait_ge(mmS, 4 * p - 12)
                issue_in(sync, p)
            # final quiesce: wait for every output DMA
            sync.wait_ge(oA[0], 16 * reps)
            sync.wait_ge(oA[1], 16 * reps)
            sync.wait_ge(oV[0], 16 * reps)
            sync.wait_ge(oV[1], 16 * reps)

        @block.tensor
        def _(tensor):
            tensor.wait_ge(wS, 16)
            for k in range(K):
                p, h = k >> 2, k & 3
                s = k & 1
                if h == 0:
                    tensor.wait_ge(iP[p % 4], 16 * (p // 4 + 1))
                psum_wait(tensor, k)
                last = None
                for ls in range(2):
                    for c in range(2):
                        dst = psp[2 * s + ls][:, c * 512 : (c + 1) * 512]
                        for a in range(2):
                            last = tensor.matmul(
                                dst,
                                wsb[:, a * 2 + c, :],
                                lat(inP[p % 4], h, ls, a),
                                start=(a == 0),
                                stop=(a == 1),
                            )
                assert last is not None
                last.then_inc(mmS, 1)

        @block.scalar
        def _(scalar):
            scalar.dma_start(out=wsb[:], in_=wall[:]).then_inc(wS, 16)
            issue_in(scalar, 0)
            for mA in range(2 * reps):  # even quads p = 2*mA
                p = 2 * mA
                b = mA & 1
                if mA >= 2:
                    scalar.wait_ge(oA[b], 16 * (mA >> 1))
                ins = None
                for h in range(4):
                    k = 4 * p + h
                    scalar.wait_ge(mmS, k + 1)
                    for ls in range(2):
                        ins = scalar.copy(
                            lat_blk(stgA[b], h, ls),
                            psp[2 * (k & 1) + ls][:].rearrange(
                                "p (c l r) -> p l c r", c=2, r=R
                            ),
                        )
                    assert ins is not None
                    ins.then_inc(eA, 1)
                # the wait makes the staging writes visible before the
                # out-DMA doorbell fires (DGE reads SBUF asynchronously)
                scalar.wait_ge(eA, _cnt_evac(4 * p + 3, True))
                scalar.dma_start(
                    out=opk_flat[prows(p), pJ(p)], in_=stgA[b][:]
                ).then_inc(oA[b], 16)

        @block.vector
        def _(vector):
            for mV in range(2 * reps):  # odd quads p = 2*mV + 1
                p = 2 * mV + 1
                b = mV & 1
                if mV >= 2:
                    vector.wait_ge(oV[b], 16 * (mV >> 1))
                ins = None
                for h in range(4):
                    k = 4 * p + h
                    vector.wait_ge(mmS, k + 1)
                    for ls in range(2):
                        ins = vector.tensor_copy(
                            lat_blk(stgV[b], h, ls),
                            psp[2 * (k & 1) + ls][:].rearrange(
                                "p (c l r) -> p l c r", c=2, r=R
                            ),
                        )
                    assert ins is not None
                    ins.then_inc(eV, 1)

        @block.gpsimd
        def _(gpsimd):
            for q in range(reps):
                pin = 4 * q + 3
                if q >= 1:
                    # inP[3] free once mm of quad pin-4 is done
                    gpsimd.wait_ge(mmS, 4 * pin - 12)
                issue_in(gpsimd, pin)
                for p in (4 * q + 1, 4 * q + 3):
                    mV = p >> 1
                    b = mV & 1
                    # eV inc fires after DVE's staging writes drain, so the
                    # SWDGE descriptor read of stgV is safe
                    gpsimd.wait_ge(eV, _cnt_evac(4 * p + 3, False))
                    gpsimd.dma_start(
                        out=opk_flat[prows(p), pJ(p)], in_=stgV[b][:]
                    ).then_inc(oV[b], 16)

    return nc


def _get_nc():
    global _NC_CACHE
    if _NC_CACHE is None:
        _NC_CACHE = _build_program()
    return _NC_CACHE


def _host_tensors(gate_real, gate_imag):
    gr = np.asarray(gate_real, dtype=np.float32)
    gi = np.asarray(gate_imag, dtype=np.float32)
    I64 = np.eye(64, dtype=np.float32)
    ws = []
    for a in range(2):
        for c in range(2):
            g2 = np.array(
                [[gr[a, c], gi[a, c]], [-gi[a, c], gr[a, c]]], dtype=np.float32
            )
            ws.append(np.kron(I64, g2))
    wall = np.stack(ws, axis=1).astype(np.float16)  # [128 k, 4 g, 128 m]
    return np.ascontiguousarray(wall)


def _in_maps(state_real, state_imag, wall):
    # state_real/state_imag: fp16 [B, N]
    maps = []
    for i in range(NCORES):
        rows = slice(i * BC, (i + 1) * BC)
        sri = np.stack([state_real[rows], state_imag[rows]], axis=1)
        maps.append({"sri": sri, "wall": wall})
    return maps


def kernel(state_real, state_imag, gate_real, gate_imag):
    state_real = np.asarray(state_real, dtype=np.float32).astype(np.float16)
    state_imag = np.asarray(state_imag, dtype=np.float32).astype(np.float16)
    wall = _host_tensors(gate_real, gate_imag)

    nc = _get_nc()
    res = run_bass_kernel_spmd(
        nc, _in_maps(state_real, state_imag, wall), list(range(NCORES))
    )

    out = np.empty((2, B, N), dtype=np.float32)
    for i in range(NCORES):
        rows = slice(i * BC, (i + 1) * BC)
        opk = res.results[i]["opk"]  # [BC, 2, N] fp16
        out[0, rows] = opk[:, 0]
        out[1, rows] = opk[:, 1]
    return out


# revision 31
# speedup vs baseline: 1.0106x; 1.0106x over previous
"""Batched single-qubit gate application on 8 TRN2 NeuronCores.

Problem: state (B=2048, N=8192) complex (separate f32 re/im planes), apply a
2x2 complex gate G on qubit 5 (pairs at stride R=128 within 256-blocks):
    out[b, l, c, r] = sum_a state[b, l, a, r] * G[a, c],  l<32, r<128.
Returns stacked (2, B, N) f32 [re, im].

Sharding: pure data parallel over the batch dim, 256 rows/core. The host
interleaves re/im at row granularity into one [256, 2, 8192] tensor per core
so every DMA touches all 128 SBUF partitions with a 2-dim DRAM access
pattern (64-partition or 3-dim-AP DMAs are much slower).

All device-side tensor data is fp16 (host casts f32 -> fp16 in, back out):
the kernel is HBM-bandwidth-bound, so halving the bytes halves the roofline
(~94us f32 -> ~47us fp16 at ~358 GB/s/core). fp16 rounding gives ~9e-4 max
rel err vs the 2e-2 gate; PSUM accumulation stays fp32.

Per-core strategy: ALL rows go through TensorE (fp16 matmul is 1 cyc/row vs
4 for f32, so PE covers the whole core's data in ~27us). The moving operand
keeps the natural DRAM row layout (interleaved [re_b; im_b] pairs on
partitions); stationary 128x128 matrices W(a,c) = kron(I64,
[[gr, gi], [-gi, gr]]) (host-built, passed as an input) turn each matmul
into "complex-scale 64 row-pairs by G[a,c]" with PSUM accumulating the two
a-terms. Work is organized as 16 chunks/rep of [128 rows x 2048 elems],
grouped into 8 PAIRS for DMA (1 MiB transfers -- SDMA engines only reach
line rate at ~1 MiB, and fewer DMAs means less fixed descriptor-path cost;
the 16 back-to-back matmuls per pair also keep the PE p-state hot):
  - PE: 8 matmuls per chunk (ls, c, a), PSUM set k&1 (2x2 banks,
    double-buffered at chunk granularity).
  - PSUM evacuation (f32 -> fp16 cast + (c,l,r)->(l,c,r) permute) alternates
    by pair between ScalarE (even pairs) and VectorE (odd pairs) so neither
    engine's copy throughput binds.
  - DMA issue is spread over three descriptor-generation paths: SP HWDGE
    ring carries most input pairs, ACT HWDGE carries the even-pair outputs,
    GPSIMD SWDGE carries every 4th input pair + the odd-pair outputs. No
    DVE 2-port perf-mode ops exist in the program, so SWDGE descriptor
    generation is never locked out of the shared SBUF port pair.

reps>1 builds the same pipeline repeated back-to-back in one NEFF (sems
keep counting) -- used only for steady-state hardware timing measurements.
"""

import sys

sys.path.insert(0, "/opt/trn_rl_repo")

from contextlib import ExitStack

import numpy as np

import concourse.bass as bass
import concourse.mybir as mybir
from concourse.bass_utils import run_bass_kernel_spmd

F32 = mybir.dt.float32
F16 = mybir.dt.float16

NCORES = 8
B = 2048
N = 8192
BC = B // NCORES  # 256 rows per core
JC = 2048  # j-chunk (elems per partition line)
NJ = N // JC  # 4
L_CHUNK = JC // 256  # 8 l-values per chunk
R = 128

_NC_CACHE = None


def _evac_is_act(k):
    # chunk k's PSUM evacuation engine: pairs alternate ACT / DVE
    return ((k >> 1) & 1) == 0


def _cnt_evac(k, act):
    # chunks j <= k evacuated by ACT (act=True) or DVE
    return sum(1 for j in range(k + 1) if _evac_is_act(j) == act)


def _build_program(reps=1):
    nc = bass.Bass()

    # sri/opk are stored in PAIR-BLOCK layout: block b = (row-group g=b>>1,
    # j-half jh=b&1) holds flat rows 128g..128g+128 x elems 4096jh..+4096 of
    # the [(BC 2), N] row-interleaved view, so every 1 MiB pair DMA is one
    # fully contiguous DRAM region. The host permutes on the way in/out.
    sri = nc.declare_dram_parameter("sri", [8, 128, 2 * JC], F16, isOutput=False)
    wall = nc.declare_dram_parameter("wall", [128, 4, 128], F16, isOutput=False)
    opk = nc.declare_dram_parameter("opk", [8, 128, 2 * JC], F16, isOutput=True)

    # SBUF. DMAs move PAIRS of chunks (1 MiB) for SDMA efficiency; compute
    # and PSUM stay at chunk granularity (a chunk's matmul output is half of
    # PSUM). 4-deep input pair ring; buffer 3 (and sem iP3) is exclusively
    # GPSIMD's -- a SWDGE DMA's completion sem must be 0 at its first use,
    # so SWDGE and HWDGE incs can't share a sem.
    wsb = nc.alloc_sbuf_tensor("wsb", [128, 4, 128], F16)
    inP = [nc.alloc_sbuf_tensor(f"inP{s}", [128, 2 * JC], F16) for s in range(4)]
    stgA = [nc.alloc_sbuf_tensor(f"stgA{s}", [128, 2 * JC], F16) for s in range(2)]
    stgV = [nc.alloc_sbuf_tensor(f"stgV{s}", [128, 2 * JC], F16) for s in range(2)]
    # PSUM: 4 tensors x 2 banks = 8 banks; chunk k uses pair-set k&1.
    psp = [nc.alloc_psum_tensor(f"ps{i}", [128, 1024], F32) for i in range(4)]

    K = 16 * reps  # chunks
    P = 8 * reps  # pairs; pair p = chunks 2p, 2p+1 (same row group)

    # moving-operand sub-lattice APs on half h of an input pair tile
    def lat(t, h, ls, a):
        return t[:].rearrange(
            "p (h ls l a r) -> p h ls l a r", h=2, ls=2, l=L_CHUNK // 2, a=2, r=R
        )[:, h, ls, :, a, :]

    # half h, ls-block of staging as [128, l, c, r] (for paired evacuation)
    def lat_blk(t, h, ls):
        return t[:].rearrange(
            "p (h ls l a r) -> p h ls l a r", h=2, ls=2, l=L_CHUNK // 2, a=2, r=R
        )[:, h, ls, :, :, :]

    with ExitStack() as _ctx:
        block = _ctx.enter_context(nc.Block())
        sem = {
            n: _ctx.enter_context(nc.semaphore(n))
            for n in [
                "wS", "mmS", "eA", "eV",
                "iP0", "iP1", "iP2", "iP3", "oA0", "oA1", "oV0", "oV1",
            ]
        }
        wS, mmS, eA, eV = sem["wS"], sem["mmS"], sem["eA"], sem["eV"]
        iP = [sem["iP0"], sem["iP1"], sem["iP2"], sem["iP3"]]
        oA = [sem["oA0"], sem["oA1"]]
        oV = [sem["oV0"], sem["oV1"]]

        def issue_in(eng, p):
            s = p % 4
            eng.dma_start(out=inP[s][:], in_=sri[p % 8]).then_inc(iP[s], 16)

        def psum_wait(eng, k):
            # psum pair-set k&1 free once evac of chunk k-2 is done
            if k >= 2:
                act = _evac_is_act(k - 2)
                eng.wait_ge(eA if act else eV, _cnt_evac(k - 2, act))

        @block.sync
        def _(sync):
            issue_in(sync, 1)
            issue_in(sync, 2)
            for p in range(4, P):
                if p % 4 == 3:
                    continue  # gpsimd's input pair
                # inP[p%4] is free once mm of pair p-4 is done
                sync.wait_ge(mmS, 2 * p - 6)
                issue_in(sync, p)
            # final quiesce: wait for every output DMA
            sync.wait_ge(oA[0], 16 * 2 * reps)
            sync.wait_ge(oA[1], 16 * 2 * reps)
            sync.wait_ge(oV[0], 16 * 2 * reps)
            sync.wait_ge(oV[1], 16 * 2 * reps)

        @block.tensor
        def _(tensor):
            tensor.wait_ge(wS, 16)
            for k in range(K):
                p, h = k >> 1, k & 1
                s = k & 1
                if h == 0:
                    tensor.wait_ge(iP[p % 4], 16 * (p // 4 + 1))
                psum_wait(tensor, k)
                last = None
                for ls in range(2):
                    for c in range(2):
                        dst = psp[2 * s + ls][:, c * 512 : (c + 1) * 512]
                        for a in range(2):
                            last = tensor.matmul(
                                dst,
                                wsb[:, a * 2 + c, :],
                                lat(inP[p % 4], h, ls, a),
                                start=(a == 0),
                                stop=(a == 1),
                            )
                assert last is not None
                last.then_inc(mmS, 1)

        @block.scalar
        def _(scalar):
            scalar.dma_start(out=wsb[:], in_=wall[:]).then_inc(wS, 16)
            issue_in(scalar, 0)
            for mA in range(4 * reps):  # even pairs p = 2*mA
                p = 2 * mA
                b = mA & 1
                if mA >= 2:
                    scalar.wait_ge(oA[b], 16 * (mA >> 1))
                ins = None
                for h in range(2):
                    k = 2 * p + h
                    scalar.wait_ge(mmS, k + 1)
                    for ls in range(2):
                        ins = scalar.copy(
                            lat_blk(stgA[b], h, ls),
                            psp[2 * (k & 1) + ls][:].rearrange(
                                "p (c l r) -> p l c r", c=2, r=R
                            ),
                        )
                    assert ins is not None
                    ins.then_inc(eA, 1)
                # the wait makes the staging writes visible before the
                # out-DMA doorbell fires (DGE reads SBUF asynchronously)
                scalar.wait_ge(eA, _cnt_evac(2 * p + 1, True))
                scalar.dma_start(out=opk[p % 8], in_=stgA[b][:]).then_inc(
                    oA[b], 16
                )

        @block.vector
        def _(vector):
            for mV in range(4 * reps):  # odd pairs p = 2*mV + 1
                p = 2 * mV + 1
                b = mV & 1
                if mV >= 2:
                    vector.wait_ge(oV[b], 16 * (mV >> 1))
                ins = None
                for h in range(2):
                    k = 2 * p + h
                    vector.wait_ge(mmS, k + 1)
                    for ls in range(2):
                        ins = vector.tensor_copy(
                            lat_blk(stgV[b], h, ls),
                            psp[2 * (k & 1) + ls][:].rearrange(
                                "p (c l r) -> p l c r", c=2, r=R
                            ),
                        )
                    assert ins is not None
                    ins.then_inc(eV, 1)

        @block.gpsimd
        def _(gpsimd):
            for q in range(2 * reps):
                pin = 4 * q + 3
                if q >= 1:
                    # inP[3] free once mm of pair pin-4 is done
                    gpsimd.wait_ge(mmS, 2 * pin - 6)
                issue_in(gpsimd, pin)
                for p in (4 * q + 1, 4 * q + 3):
                    mV = p >> 1
                    b = mV & 1
                    # eV inc fires after DVE's staging writes drain, so the
                    # SWDGE descriptor read of stgV is safe
                    gpsimd.wait_ge(eV, _cnt_evac(2 * p + 1, False))
                    gpsimd.dma_start(out=opk[p % 8], in_=stgV[b][:]).then_inc(
                        oV[b], 16
                    )

    return nc


def _get_nc():
    global _NC_CACHE
    if _NC_CACHE is None:
        _NC_CACHE = _build_program()
    return _NC_CACHE


def _host_tensors(gate_real, gate_imag):
    gr = np.asarray(gate_real, dtype=np.float32)
    gi = np.asarray(gate_imag, dtype=np.float32)
    I64 = np.eye(64, dtype=np.float32)
    ws = []
    for a in range(2):
        for c in range(2):
            g2 = np.array(
                [[gr[a, c], gi[a, c]], [-gi[a, c], gr[a, c]]], dtype=np.float32
            )
            ws.append(np.kron(I64, g2))
    wall = np.stack(ws, axis=1).astype(np.float16)  # [128 k, 4 g, 128 m]
    return np.ascontiguousarray(wall)


def _in_maps(state_real, state_imag, wall):
    # state_real/state_imag: fp16 [B, N]. Build the pair-block layout:
    # flat row-interleaved [512, 8192] -> [4 g, 128 row, 2 jh, 4096]
    # -> [8 (g jh), 128, 4096] so each DMA pair block is contiguous.
    maps = []
    for i in range(NCORES):
        rows = slice(i * BC, (i + 1) * BC)
        flat = np.stack([state_real[rows], state_imag[rows]], axis=1).reshape(
            2 * BC, N
        )
        sri = np.ascontiguousarray(
            flat.reshape(4, 128, 2, 2 * JC).transpose(0, 2, 1, 3)
        ).reshape(8, 128, 2 * JC)
        maps.append({"sri": sri, "wall": wall})
    return maps


def kernel(state_real, state_imag, gate_real, gate_imag):
    state_real = np.asarray(state_real, dtype=np.float32).astype(np.float16)
    state_imag = np.asarray(state_imag, dtype=np.float32).astype(np.float16)
    wall = _host_tensors(gate_real, gate_imag)

    nc = _get_nc()
    res = run_bass_kernel_spmd(
        nc, _in_maps(state_real, state_imag, wall), list(range(NCORES))
    )

    out = np.empty((2, B, N), dtype=np.float32)
    for i in range(NCORES):
        rows = slice(i * BC, (i + 1) * BC)
        opk = res.results[i]["opk"]  # [8, 128, 4096] fp16 pair blocks
        flat = (
            opk.reshape(4, 2, 128, 2 * JC)
            .transpose(0, 2, 1, 3)
            .reshape(2 * BC, N)
        )
        out[0, rows] = flat[0::2]
        out[1, rows] = flat[1::2]
    return out
